# revision 15
# baseline (speedup 1.0000x reference)
"""Trainium2 Bass kernel for ConstrainedAttentionModel (sparse_attention).

Full-input contract: kernel(x=[8,2048] int, C=[4,4] f32) -> [8,2048] f32.
Data parallel across 8 NeuronCores: one batch row per core.

Math (per row, T=2048, k=4, V=2048):
  scores[t] = sum_{i,j} C[i,j] * [x[t-j] == x[T-1-i]]   (t-j >= 0)
  scores[T-1] = -1e9; attn = softmax(scores)
  out[v] = sum_t attn[t] * [x[t] == v]

Device strategy (t = 16p + f layout on 128 partitions):
  - one contiguous DMA loads a 19-token window per partition; the 4
    shifted (lag j) copies are overlapping SBUF *views* (stride -1 on j)
  - a 256-byte const row (q replicated, C, lo/hi iota rows, fp16-packed)
    lands on one partition and is broadcast to all 128 via a single K=1
    PE matmul; one PSUM->SBUF fp16 copy fans it out
  - the equality/score chain runs in fp16 (packed 2x DVE mode)
  - exp on the scalar engine -> E fp16, f-halves pipelined
  - vocab one-hot factorized as v = 64*hi + lo: A[f,hi]=E[f]*[x>>6==hi],
    B[f,lo]=[x&63==lo]; out[hi,lo] = 16 PSUM-accumulated fp16 matmuls
    with contiguous operands (full PE streaming rate)
  - the t=T-1 (softmax-masked) position is excluded by contracting only
    127 partitions in the last matmul
  - the kernel returns the UNNORMALIZED histogram; softmax
    normalization happens on host: out = y / y.sum() (Z == sum(y))
"""
import os
import numpy as np
import concourse.bass as bass
import concourse.bacc as bacc
import concourse.tile as tile
from concourse import mybir

P = 128
T = 2048
F = T // P  # 16
K = 4
FH = F // 2  # 8
NHI = 32
NLO = 64
XW = F + K - 1  # 19

fp32 = mybir.dt.float32
fp16 = mybir.dt.float16
i32 = mybir.dt.int32
Alu = mybir.AluOpType
Act = mybir.ActivationFunctionType

N_WARM1 = int(os.environ.get("KERNEL_N_WARM1", "14"))
N_WARM2 = int(os.environ.get("KERNEL_N_WARM2", "22"))

B = 8


def _build_nc():
    nc = bacc.Bacc()
    xin = nc.dram_tensor("xin", [K - 1 + T], i32, kind="ExternalInput")
    crow = nc.dram_tensor("crow", [64], fp32, kind="ExternalInput")
    y = nc.dram_tensor("y", [T], fp32, kind="ExternalOutput")

    with tile.TileContext(nc) as tc:
        with (
            tc.tile_pool(name="sb", bufs=1) as sb,
            tc.tile_pool(name="ps", bufs=1, space="PSUM") as ps,
        ):
            XF = sb.tile([P, XW], i32)  # XF[p,e] = x[16p+e-3], pad -1
            CROW = sb.tile([1, 64], fp32)  # 128 fp16: q, C, IL, IH

            nc.sync.dma_start(
                out=XF[:],
                in_=bass.AP(tensor=xin[:].tensor, offset=0, ap=[[F, P], [1, XW]]),
            )
            nc.scalar.dma_start(out=CROW[:], in_=crow[None, :])

            ONESH = sb.tile([1, P], fp16)
            nc.vector.memset(ONESH[:], 1.0)
            c1 = nc.const_aps.aps[(fp32, 1.0)]

            # PE warm-up: narrow matmuls keep the HAM clock gate open
            warm = ps.tile([1, 1], fp32)
            for w in range(N_WARM1):
                nc.tensor.matmul(
                    warm[:], lhsT=c1[:, 0:1], rhs=c1[:, 0:1], start=True,
                    stop=True, skip_group_check=True,
                )

            # broadcast the const row to all partitions via one K=1 matmul
            CB = ps.tile([P, 128], fp32)
            nc.tensor.matmul(
                CB[:],
                lhsT=ONESH[:],
                rhs=CROW[:].bitcast(fp16),
                start=True,
                stop=True,
                skip_group_check=True,
            )

            for w in range(N_WARM2):
                nc.tensor.matmul(
                    warm[:], lhsT=c1[:, 0:1], rhs=c1[:, 0:1], start=True,
                    stop=True, skip_group_check=True,
                )

            # fp16 token window + lo/hi parts
            XF16 = sb.tile([P, XW], fp16)
            nc.vector.tensor_copy(out=XF16[:], in_=XF[:])
            X0 = XF[:, K - 1 : K - 1 + F]
            X016 = XF16[:, K - 1 : K - 1 + F]
            XLH = sb.tile([P, F], i32)
            nc.vector.tensor_scalar(
                out=XLH[:], in0=X0, scalar1=63, scalar2=None,
                op0=Alu.bitwise_and,
            )
            XLOHI = sb.tile([P, 2 * F], fp16)
            XLO = XLOHI[:, 0:F]
            XHI = XLOHI[:, F : 2 * F]
            nc.vector.tensor_copy(out=XLO, in_=XLH[:])
            nc.vector.tensor_tensor(
                out=XHI, in0=X016, in1=XLO, op=Alu.subtract
            )

            # q + C to every partition, fp16 (iota rows fanned out later,
            # off the EQ critical path)
            CBQ = sb.tile([P, 128], fp16)
            nc.vector.tensor_copy(out=CBQ[:, 0:32], in_=CB[:, 0:32])
            Q16 = CBQ[:, 0:16].rearrange("p (i j) -> p i j", j=K)
            CN16 = CBQ[:, 16:32]
            IL16 = CBQ[:, 32:96]
            IH16 = CBQ[:, 96:128]

            EQ = sb.tile([P, F, K, K], fp16)
            CE = sb.tile([P, F, 16], fp16)
            SC = sb.tile([P, F], fp16)
            E = sb.tile([P, F], fp16)
            AEQ = sb.tile([P, NHI, F], fp16)  # transposed: [hi, f]
            Bt = sb.tile([P, F, NLO], fp16)
            A = sb.tile([P, NHI, F], fp16)  # transposed: lhsT slice per f
            acc = ps.tile([NHI, NLO], fp32)

            # EQ[p,f,i,j] = [x[t-j] == q_i]  (t = 16p+f)
            sub = XF16[:, K - 1 :][:]
            XWIN = bass.AP(
                tensor=sub.tensor,
                offset=sub.offset,
                ap=[sub.ap[0], [1, F], [0, K], [-1, K]],
            )
            nc.vector.tensor_tensor(
                out=EQ[:],
                in0=XWIN,
                in1=Q16[:, None, :, :].broadcast_to([P, F, K, K]),
                op=Alu.is_equal,
            )
            nc.vector.tensor_tensor(
                out=CE[:],
                in0=EQ[:].rearrange("p f i j -> p f (i j)"),
                in1=CN16[:, None, :].broadcast_to([P, F, 16]),
                op=Alu.mult,
            )
            with nc.allow_low_precision(reason="16 products of |C|<0.1 each"):
                for h in range(2):
                    fs = slice(h * FH, (h + 1) * FH)
                    nc.vector.reduce_sum(
                        out=SC[:, fs], in_=CE[:, fs], axis=mybir.AxisListType.X
                    )
                    nc.scalar.activation(
                        out=E[:, fs], in_=SC[:, fs], func=Act.Exp
                    )

            # fan out the iota rows (EQ chain no longer needs CBQ)
            nc.vector.tensor_copy(out=CBQ[:, 32:128], in_=CB[:, 32:128])

            for h in range(2):
                fs = slice(h * FH, (h + 1) * FH)
                nc.vector.tensor_tensor(
                    out=Bt[:, fs],
                    in0=XLO[:, fs, None].broadcast_to([P, FH, NLO]),
                    in1=IL16[:, None, :].broadcast_to([P, FH, NLO]),
                    op=Alu.is_equal,
                )
                nc.vector.tensor_tensor(
                    out=AEQ[:, :, fs],
                    in0=XHI[:, None, fs].broadcast_to([P, NHI, FH]),
                    in1=IH16[:, :, None].broadcast_to([P, NHI, FH]),
                    op=Alu.is_equal,
                )
                nc.vector.tensor_tensor(
                    out=A[:, :, fs],
                    in0=AEQ[:, :, fs],
                    in1=E[:, None, fs].broadcast_to([P, NHI, FH]),
                    op=Alu.mult,
                )
                for f in range(h * FH, (h + 1) * FH):
                    # t=2047 (p=127, f=15) is excluded from the contraction
                    # entirely -> attn[T-1] = 0 and Z skips it
                    pe = P - 1 if f == F - 1 else P
                    nc.tensor.matmul(
                        acc[:],
                        lhsT=A[0:pe, :, f],
                        rhs=Bt[0:pe, f, :],
                        start=(f == 0),
                        stop=(f == F - 1),
                        skip_group_check=True,
                    )

            OUT = sb.tile([NHI, NLO], fp32)
            nc.vector.tensor_copy(out=OUT[:], in_=acc[:])
            yv = y[:].rearrange("(h l) -> h l", l=NLO)
            nc.sync.dma_start(out=yv[0:16], in_=OUT[0:16, :])
            nc.scalar.dma_start(out=yv[16:32], in_=OUT[16:32, :])
    nc.compile()
    return nc


def _make_crow(x_row: np.ndarray, C: np.ndarray) -> np.ndarray:
    cw = np.zeros(128, np.float16)
    q = x_row[T - 1 : T - 1 - K : -1].astype(np.float16)  # q[i] = x[T-1-i]
    cw[0:16] = np.repeat(q, K)  # q[i] at 4i+j
    cw[16:32] = C.reshape(16).astype(np.float16)  # C[i,j] at 4i+j
    cw[32:96] = np.arange(NLO, dtype=np.float16)
    cw[96:128] = 64.0 * np.arange(NHI, dtype=np.float16)
    return cw.view(np.float32)


def _host_prep(x_row: np.ndarray, C: np.ndarray):
    x_row = x_row.astype(np.int32)
    xin = np.concatenate([np.full(K - 1, -1, np.int32), x_row])
    return {"xin": xin, "crow": _make_crow(x_row, C)}


_NC_CACHE = {}


def _get_nc():
    if "nc" not in _NC_CACHE:
        _NC_CACHE["nc"] = _build_nc()
    return _NC_CACHE["nc"]


def kernel(x: np.ndarray, C: np.ndarray, _spmd_kwargs: dict | None = None):
    from concourse.bass_utils import run_bass_kernel_spmd

    x = np.asarray(x).astype(np.int32)  # token ids < 2048, exact
    C = np.asarray(C).astype(np.float32)
    assert x.shape == (B, T) and C.shape == (K, K)
    in_maps = [_host_prep(x[b], C) for b in range(B)]
    res = run_bass_kernel_spmd(
        _get_nc(), in_maps, core_ids=list(range(B)), **(_spmd_kwargs or {})
    )
    # y is the unnormalized E-weighted vocab histogram; Z == y.sum()
    hist = np.stack([res.results[b]["y"] for b in range(B)], axis=0)
    out = (hist / hist.sum(axis=1, keepdims=True)).astype(np.float32)
    if _spmd_kwargs:
        kernel.last_results = res
    return out


# revision 20
# speedup vs baseline: 1.0040x; 1.0040x over previous
"""Trainium2 Bass kernel for ConstrainedAttentionModel (sparse_attention).

Full-input contract: kernel(x=[8,2048] int, C=[4,4] f32) -> [8,2048] f32.
Data parallel across 8 NeuronCores: one batch row per core.

Math (per row, T=2048, k=4, V=2048):
  scores[t] = sum_{i,j} C[i,j] * [x[t-j] == x[T-1-i]]   (t-j >= 0)
  scores[T-1] = -1e9; attn = softmax(scores)
  out[v] = sum_t attn[t] * [x[t] == v]

Device strategy (t = 16p + f layout on 128 partitions):
  - one contiguous DMA loads a 19-token window per partition; the 4
    shifted (lag j) copies are overlapping SBUF *views* (stride -1 on j)
  - a 256-byte const row (q replicated, C, lo/hi iota rows, fp16-packed)
    lands on one partition and is broadcast to all 128 via a single K=1
    PE matmul; one PSUM->SBUF fp16 copy fans it out
  - the equality/score chain runs in fp16 (packed 2x DVE mode)
  - exp on the scalar engine -> E fp16, f-halves pipelined
  - vocab one-hot factorized as v = 64*hi + lo: A[f,hi]=E[f]*[x>>6==hi],
    B[f,lo]=[x&63==lo]; out[hi,lo] = 16 PSUM-accumulated fp16 matmuls
    with contiguous operands (full PE streaming rate)
  - the t=T-1 (softmax-masked) position is excluded by contracting only
    127 partitions in the last matmul
  - the kernel returns the UNNORMALIZED histogram; softmax
    normalization happens on host: out = y / y.sum() (Z == sum(y))
"""
import os
import numpy as np
import concourse.bass as bass
import concourse.bacc as bacc
import concourse.tile as tile
from concourse import mybir

P = 128
T = 2048
F = T // P  # 16
K = 4
FH = F // 2  # 8
NHI = 32
NLO = 64
XW = F + K - 1  # 19

fp32 = mybir.dt.float32
fp16 = mybir.dt.float16
i32 = mybir.dt.int32
i16 = mybir.dt.int16
Alu = mybir.AluOpType
Act = mybir.ActivationFunctionType

N_WARM1 = int(os.environ.get("KERNEL_N_WARM1", "14"))
N_WARM2 = int(os.environ.get("KERNEL_N_WARM2", "22"))

B = 8


def _build_nc():
    nc = bacc.Bacc()
    xin = nc.dram_tensor("xin", [K - 1 + T], i16, kind="ExternalInput")
    crow = nc.dram_tensor("crow", [64], fp32, kind="ExternalInput")
    y = nc.dram_tensor("y", [T], fp32, kind="ExternalOutput")

    with tile.TileContext(nc) as tc:
        with (
            tc.tile_pool(name="sb", bufs=1) as sb,
            tc.tile_pool(name="ps", bufs=1, space="PSUM") as ps,
        ):
            XF = sb.tile([P, XW], i16)  # XF[p,e] = x[16p+e-3], pad -1
            CROW = sb.tile([1, 64], fp32)  # 128 fp16: q, C, IL, IH

            nc.sync.dma_start(
                out=XF[:],
                in_=bass.AP(tensor=xin[:].tensor, offset=0, ap=[[F, P], [1, XW]]),
            )
            nc.scalar.dma_start(out=CROW[:], in_=crow[None, :])

            ONESH = sb.tile([1, P], fp16)
            nc.vector.memset(ONESH[:], 1.0)
            c1 = nc.const_aps.aps[(fp32, 1.0)]

            # PE warm-up: narrow matmuls keep the HAM clock gate open
            warm = ps.tile([1, 1], fp32)
            for w in range(N_WARM1):
                nc.tensor.matmul(
                    warm[:], lhsT=c1[:, 0:1], rhs=c1[:, 0:1], start=True,
                    stop=True, skip_group_check=True,
                )

            # broadcast the const row to all partitions via one K=1 matmul
            CB = ps.tile([P, 128], fp32)
            nc.tensor.matmul(
                CB[:],
                lhsT=ONESH[:],
                rhs=CROW[:].bitcast(fp16),
                start=True,
                stop=True,
                skip_group_check=True,
            )

            for w in range(N_WARM2):
                nc.tensor.matmul(
                    warm[:], lhsT=c1[:, 0:1], rhs=c1[:, 0:1], start=True,
                    stop=True, skip_group_check=True,
                )

            # fp16 token window + lo/hi parts
            XF16 = sb.tile([P, XW], fp16)
            nc.vector.tensor_copy(out=XF16[:], in_=XF[:])
            X0 = XF[:, K - 1 : K - 1 + F]
            X016 = XF16[:, K - 1 : K - 1 + F]
            XLH = sb.tile([P, F], i16)
            nc.vector.tensor_scalar(
                out=XLH[:], in0=X0, scalar1=63, scalar2=None,
                op0=Alu.bitwise_and,
            )
            XLOHI = sb.tile([P, 2 * F], fp16)
            XLO = XLOHI[:, 0:F]
            XHI = XLOHI[:, F : 2 * F]
            nc.vector.tensor_copy(out=XLO, in_=XLH[:])
            nc.vector.tensor_tensor(
                out=XHI, in0=X016, in1=XLO, op=Alu.subtract
            )

            # q + C to every partition, fp16 (iota rows fanned out later,
            # off the EQ critical path)
            CBQ = sb.tile([P, 128], fp16)
            nc.vector.tensor_copy(out=CBQ[:, 0:32], in_=CB[:, 0:32])
            Q16 = CBQ[:, 0:16].rearrange("p (i j) -> p i j", j=K)
            CN16 = CBQ[:, 16:32]
            IL16 = CBQ[:, 32:96]
            IH16 = CBQ[:, 96:128]

            EQ = sb.tile([P, F, K, K], fp16)
            CE = sb.tile([P, F, 16], fp16)
            SC = sb.tile([P, F], fp16)
            E = sb.tile([P, F], fp16)
            AEQ = sb.tile([P, NHI, F], fp16)  # transposed: [hi, f]
            Bt = sb.tile([P, F, NLO], fp16)
            A = sb.tile([P, NHI, F], fp16)  # transposed: lhsT slice per f
            acc = ps.tile([NHI, NLO], fp32)

            # EQ[p,f,i,j] = [x[t-j] == q_i]  (t = 16p+f)
            sub = XF16[:, K - 1 :][:]
            XWIN = bass.AP(
                tensor=sub.tensor,
                offset=sub.offset,
                ap=[sub.ap[0], [1, F], [0, K], [-1, K]],
            )
            with tc.high_priority():
                nc.vector.tensor_tensor(
                    out=EQ[:],
                    in0=XWIN,
                    in1=Q16[:, None, :, :].broadcast_to([P, F, K, K]),
                    op=Alu.is_equal,
                )
                nc.vector.tensor_tensor(
                    out=CE[:],
                    in0=EQ[:].rearrange("p f i j -> p f (i j)"),
                    in1=CN16[:, None, :].broadcast_to([P, F, 16]),
                    op=Alu.mult,
                )
                with nc.allow_low_precision(reason="16 products of |C|<0.1"):
                    for h in range(2):
                        fs = slice(h * FH, (h + 1) * FH)
                        nc.vector.reduce_sum(
                            out=SC[:, fs], in_=CE[:, fs],
                            axis=mybir.AxisListType.X,
                        )
                        nc.scalar.activation(
                            out=E[:, fs], in_=SC[:, fs], func=Act.Exp
                        )

            # fan out the iota rows on the idle scalar engine
            nc.scalar.activation(
                out=CBQ[:, 32:128], in_=CB[:, 32:128], func=Act.Identity
            )

            for h in range(2):
                fs = slice(h * FH, (h + 1) * FH)
                nc.vector.tensor_tensor(
                    out=Bt[:, fs],
                    in0=XLO[:, fs, None].broadcast_to([P, FH, NLO]),
                    in1=IL16[:, None, :].broadcast_to([P, FH, NLO]),
                    op=Alu.is_equal,
                )
                nc.vector.tensor_tensor(
                    out=AEQ[:, :, fs],
                    in0=XHI[:, None, fs].broadcast_to([P, NHI, FH]),
                    in1=IH16[:, :, None].broadcast_to([P, NHI, FH]),
                    op=Alu.is_equal,
                )
                nc.vector.tensor_tensor(
                    out=A[:, :, fs],
                    in0=AEQ[:, :, fs],
                    in1=E[:, None, fs].broadcast_to([P, NHI, FH]),
                    op=Alu.mult,
                )
                for f in range(h * FH, (h + 1) * FH):
                    # t=2047 (p=127, f=15) is excluded from the contraction
                    # entirely -> attn[T-1] = 0 and Z skips it
                    pe = P - 1 if f == F - 1 else P
                    nc.tensor.matmul(
                        acc[:],
                        lhsT=A[0:pe, :, f],
                        rhs=Bt[0:pe, f, :],
                        start=(f == 0),
                        stop=(f == F - 1),
                        skip_group_check=True,
                    )

            OUT = sb.tile([NHI, NLO], fp32)
            nc.vector.tensor_copy(out=OUT[:], in_=acc[:])
            yv = y[:].rearrange("(h l) -> h l", l=NLO)
            nc.sync.dma_start(out=yv[0:16], in_=OUT[0:16, :])
            nc.scalar.dma_start(out=yv[16:32], in_=OUT[16:32, :])
    nc.compile()
    return nc


def _make_crow(x_row: np.ndarray, C: np.ndarray) -> np.ndarray:
    cw = np.zeros(128, np.float16)
    q = x_row[T - 1 : T - 1 - K : -1].astype(np.float16)  # q[i] = x[T-1-i]
    cw[0:16] = np.repeat(q, K)  # q[i] at 4i+j
    cw[16:32] = C.reshape(16).astype(np.float16)  # C[i,j] at 4i+j
    cw[32:96] = np.arange(NLO, dtype=np.float16)
    cw[96:128] = 64.0 * np.arange(NHI, dtype=np.float16)
    return cw.view(np.float32)


def _host_prep(x_row: np.ndarray, C: np.ndarray):
    x_row = x_row.astype(np.int16)
    xin = np.concatenate([np.full(K - 1, -1, np.int16), x_row])
    return {"xin": xin, "crow": _make_crow(x_row, C)}


_NC_CACHE = {}


def _get_nc():
    if "nc" not in _NC_CACHE:
        _NC_CACHE["nc"] = _build_nc()
    return _NC_CACHE["nc"]


def kernel(x: np.ndarray, C: np.ndarray, _spmd_kwargs: dict | None = None):
    from concourse.bass_utils import run_bass_kernel_spmd

    x = np.asarray(x).astype(np.int32)  # token ids < 2048, exact
    C = np.asarray(C).astype(np.float32)
    assert x.shape == (B, T) and C.shape == (K, K)
    in_maps = [_host_prep(x[b], C) for b in range(B)]
    res = run_bass_kernel_spmd(
        _get_nc(), in_maps, core_ids=list(range(B)), **(_spmd_kwargs or {})
    )
    # y is the unnormalized E-weighted vocab histogram; Z == y.sum()
    hist = np.stack([res.results[b]["y"] for b in range(B)], axis=0)
    out = (hist / hist.sum(axis=1, keepdims=True)).astype(np.float32)
    if _spmd_kwargs:
        kernel.last_results = res
    return out
